# revision 3
# baseline (speedup 1.0000x reference)
"""Causal self-attention Bass/Tile kernel for Trainium2, 8 NeuronCores SPMD.

Problem: B=4, T=2048, C=1024, H=16 heads, D=64, f32 in/out.
    qkv = x @ w_qkv.T; per-head causal softmax(q k^T / sqrt(D)) @ v;
    out = attn @ w_out.T + b_out.

Sharding (hybrid batch x tensor-parallel): core c handles batch b = c//2 and
head group hg = c%2 (8 of 16 heads). Each core computes a full [T, C] partial
of the output projection restricted to its heads; the host sums the two
partials per batch and adds the bias.

v2 design (vs the v1 345us baseline):
  - scores computed TRANSPOSED per l-block: scT[l, i] = k_h q_h^T, exp on
    ScalarE in wide (<=1024) chunks, diagonal causal mask on DVE.  All 16
    l-blocks of a head stay resident (compact [P, T-l0] bf16 tiles).
  - PV in [i, d] orientation, ib-major: for each i-block, one PSUM tile
    [P, 65] accumulates ex[l, i]^T @ [v | 1] over l-blocks 0..ib in a single
    burst.  Full 128-partition outputs halve the PE column count vs the v1
    [d, i] orientation, the PSUM accumulator is 1 bank (so the scores pool
    gets depth 3), and the softmax denominator lands per-PARTITION (col 64):
    normalization is an IEEE DVE reciprocal + tensor_scalar mul (no
    DRAM-bounce partition broadcast).
  - attn -> attnT via XBAR DMA transpose (idle DMA engines), feeding the
    output projection, which is interleaved into head 7's stream.
  - DMA order and qk emission tuned so the first exp starts early.
"""

import sys

if "/opt/trn_rl_repo" not in sys.path:
    sys.path.insert(0, "/opt/trn_rl_repo")

import numpy as np
import ml_dtypes

import concourse.bass as bass
import concourse.tile as tile
import concourse.mybir as mybir
from concourse import bacc
from concourse.bass_utils import run_bass_kernel_spmd

BF16 = mybir.dt.bfloat16
F32 = mybir.dt.float32
NPBF16 = ml_dtypes.bfloat16
EXPF = mybir.ActivationFunctionType.Exp

P = 128
C = 1024
CC = C // P      # 8 contraction chunks
NH = 8           # heads per core
D = 64
J = NH * D       # 512 (local q/k/v width)
JC = J // P      # 4 j-chunks


def build_program(T=2048):
    LC = T // P          # l/i 128-blocks (16)
    NS = T // 512        # 512-wide supers (4)
    SCALE = 0.125        # 1/sqrt(D)

    nc = bacc.Bacc("TRN2", target_bir_lowering=False, debug=False, num_devices=8)

    xT_d = nc.dram_tensor("xT", [CC, P, T], BF16, kind="ExternalInput")
    wqkvT_d = nc.dram_tensor("wqkvT", [CC, P, 3 * J], BF16, kind="ExternalInput")
    woutT_d = nc.dram_tensor("woutT", [JC, P, C], BF16, kind="ExternalInput")
    mask_d = nc.dram_tensor("trimask", [P, P], BF16, kind="ExternalInput")
    ident_d = nc.dram_tensor("ident", [P, P], BF16, kind="ExternalInput")
    ya_d = nc.dram_tensor("ya", [LC, P, C], BF16, kind="ExternalOutput")
    yc_d = nc.dram_tensor("yc", [LC, P, C], BF16, kind="ExternalOutput")

    with tile.TileContext(nc) as tc:
        with (
            tc.tile_pool(name="persist", bufs=1) as persist,
            tc.tile_pool(name="io", bufs=1) as io_pool,
            tc.tile_pool(name="expp", bufs=1) as exp_pool,
            tc.tile_pool(name="asbp", bufs=3) as asb_pool,
            tc.tile_pool(name="rcp", bufs=3) as rc_pool,
            tc.tile_pool(name="outp", bufs=6) as out_pool,
            tc.tile_pool(name="ps_a", bufs=2, space="PSUM") as ps_a,
            tc.tile_pool(name="ps_b", bufs=2, space="PSUM") as ps_b,
        ):
            woutT = persist.tile([P, JC, C], BF16)
            trimask = persist.tile([P, P], BF16)
            ident = persist.tile([P, P], BF16)
            qkT = persist.tile([P, 2 * JC, T], BF16)
            v_aug = persist.tile([P, LC, NH, D + 1], BF16)
            attnT = persist.tile([P, JC, T], BF16)
            warm = persist.tile([1, 8], F32)
            xT = io_pool.tile([P, CC, T], BF16)
            wqkvT = io_pool.tile([P, CC, 3 * J], BF16)

            # ---------------- input DMA, ordered for earliest first-exp ----
            # q-chunk0 and k-chunk0 weight slices, then x supers; v weights
            # land mid-stream (PV lags scores); the rest afterwards.
            def ldx(c0, c1, t0, t1):
                nc.sync.dma_start(
                    xT[:, c0:c1, t0:t1],
                    xT_d[c0:c1, :, t0:t1].rearrange("c p t -> p c t"),
                )

            def ldw(j0, j1):
                nc.sync.dma_start(
                    wqkvT[:, :, j0:j1],
                    wqkvT_d[:, :, j0:j1].rearrange("c p t -> p c t"),
                )

            ldx(0, 4, 0, 512)
            ldw(0, P)
            ldx(4, 8, 0, 512)
            ldw(J, J + P)
            nc.sync.dma_start(trimask[:], mask_d[:])
            ldx(0, 8, 512, 1024)
            ldx(0, 8, 1024, 1536)
            ldx(0, 8, 1536, 2048)
            ldw(2 * J, 3 * J)
            ldw(P, J)
            ldw(J + P, 2 * J)
            for jc in range(JC):
                nc.sync.dma_start(woutT[:, jc, :], woutT_d[jc])
            nc.sync.dma_start(ident[:], ident_d[:])

            # warm up the PE p-state ramp while the input DMAs are in flight:
            # ~5us of dependency-free matmuls on a zeroed tile so the first
            # real matmuls run at full clock.
            wsrc = persist.tile([P, 512], BF16, name="wsrc")
            nc.vector.memset(wsrc[:], 0.0)
            nc.gpsimd.memset(v_aug[:, :, :, D], 1.0)
            # warm up the exp activation table before the first real exp
            nc.gpsimd.memset(warm[:], 0.0)
            nc.scalar.activation(warm[:], warm[:], EXPF)
            for i in range(2):
                wdst = ps_a.tile([P, 512], F32, tag="fl", name=f"wdst{i}")
                for k in range(8):
                    nc.tensor.matmul(
                        wdst[:], wsrc[:, 0:P], wsrc[:], start=True, stop=True
                    )

            # ---------------- QKV projection pieces ----------------
            # fills and out-proj halves share the 1-bank "fl" rotation so the
            # scores pipeline ("sc" tag) never blocks on them.
            def emit_qk_super(jc, s, dve=False):
                """q/k chunk jc, single t-super s.  Mid-stream fills evacuate
                via the idle GpSimd so their rotation never queues behind the
                DVE's evac work; startup supers stay on DVE (critical path)."""
                pq = ps_a.tile([P, 512], F32, tag="fl", name=f"qks{jc}_{s}")
                for cc in range(CC):
                    nc.tensor.matmul(
                        pq[:],
                        wqkvT[:, cc, jc * P : (jc + 1) * P],
                        xT[:, cc, s * 512 : (s + 1) * 512],
                        start=(cc == 0),
                        stop=(cc == CC - 1),
                    )
                nc.vector.tensor_copy(qkT[:, jc, s * 512 : (s + 1) * 512], pq[:])

            def emit_qk_pair(jc, k):
                """q/k chunk jc, t-supers 2k and 2k+1."""
                for i in range(2):
                    emit_qk_super(jc, 2 * k + i)

            def emit_v_block(lc):
                """v for t-block lc into v_aug."""
                pq = ps_a.tile([P, 512], F32, tag="fl", name=f"v{lc}")
                for cc in range(CC):
                    nc.tensor.matmul(
                        pq[:],
                        xT[:, cc, lc * P : (lc + 1) * P],
                        wqkvT[:, cc, 2 * J : 3 * J],
                        start=(cc == 0),
                        stop=(cc == CC - 1),
                    )
                nc.vector.tensor_copy(
                    v_aug[:, lc, :, 0:D],
                    pq[:].rearrange("p (h d) -> p h d", d=D),
                )

            def emit_v_pair(k):
                emit_v_block(2 * k)
                emit_v_block(2 * k + 1)

            # startup: exactly what the first exp chunks need, in order
            emit_qk_super(0, 0, dve=True)    # q0 s0
            emit_qk_super(JC, 0, dve=True)   # k0 s0
            emit_qk_super(0, 1, dve=True)    # q0 s1 -> scores lb0 chunk A ready

            # deferred qkv fill: remaining startup supers + pairs 1..3 + v.
            inserts = {hh: [] for hh in range(NH)}
            inserts[0] = [
                (0, ("qks", (JC, 1))),
                (1, ("v", 0)),
                (2, ("v", 1)),
                (2, ("qks", (JC, 2))),
                (3, ("v", 2)),
                (4, ("qks", (JC, 3))),
                (5, ("v", 3)),
                (7, ("v", 4)),
                (9, ("v", 5)),
                (11, ("v", 6)),
                (13, ("v", 7)),
            ]
            for pair in range(1, JC):
                tiles_ = []
                for jc in (pair, JC + pair):
                    for k in range((NS + 1) // 2):
                        tiles_.append((jc, k))
                carriers = (1,) if pair == 1 else (2 * pair - 2, 2 * pair - 1)
                for i, tl in enumerate(tiles_):
                    hh = carriers[i % len(carriers)]
                    inserts[hh].append((None, ("qk", tl)))
            for hh in range(1, NH):
                items = inserts[hh]
                n_auto = len([it for it in items if it[0] is None])
                auto_pos = [
                    (LC * (i + 1)) // max(1, n_auto) - 1 for i in range(n_auto)
                ]
                autos = [it for it in items if it[0] is None]
                fixed = [it for it in items if it[0] is not None]
                inserts[hh] = sorted(
                    fixed + [(auto_pos[i], autos[i][1]) for i in range(len(autos))]
                )

            # ---------------- attention ----------------
            def open_ex(h, lb):
                # compact per-l-block tile covering columns [l0, T); one tag
                # per lb so all 16 stay resident for a head.  lb 0/1 are
                # double-buffered: the next head's first two score blocks are
                # emitted inside the previous head (cross-head pipelining).
                return exp_pool.tile(
                    [P, T - lb * P], BF16, tag=f"ex{lb}", name=f"ex{h}_{lb}",
                    bufs=3 if lb < 1 else (2 if lb < 2 else 1),
                )

            def emit_scores_chunk(h, lb, ex, tstart):
                """PE score matmuls + ACT exp for [tstart, tstart+1024)."""
                bp = (h % 2) * 64
                chq = h // 2
                l0 = lb * P
                sc = ps_a.tile(
                    [P, 2, 512], F32, tag="sc", name=f"sc{h}_{lb}_{tstart}"
                )
                scf = sc[:].rearrange("p a b -> p (a b)")
                lo = max(l0, tstart)
                hi = min(tstart + 1024, T)
                c0 = lo
                while c0 < hi:
                    n = min(512 - (c0 % 512), hi - c0)
                    nc.tensor.matmul(
                        scf[:, c0 - tstart : c0 - tstart + n],
                        qkT[bp : bp + 64, JC + chq, l0 : l0 + P],
                        qkT[bp : bp + 64, chq, c0 : c0 + n],
                        start=True,
                        stop=True,
                    )
                    c0 += n
                nc.scalar.activation(
                    ex[:, lo - l0 : hi - l0], scf[:, lo - tstart : hi - tstart],
                    EXPF, scale=SCALE,
                )

            def emit_mask(ex, h):
                # head 7: Pool is idle and DVE carries the out-proj copies,
                # so the mask goes to GpSimd to unblock the diag-MM LDW.
                eng = nc.gpsimd if h == NH - 1 else nc.vector
                eng.tensor_mul(ex[:, 0:P], ex[:, 0:P], trimask[:])

            def emit_scores_exp(h, lb):
                l0 = lb * P
                ex = open_ex(h, lb)
                for tstart in range((l0 // 1024) * 1024, T, 1024):
                    emit_scores_chunk(h, lb, ex, tstart)
                return ex

            def emit_pv_burst(h, ib, exs, rc, asb):
                """One i-block: accumulate over l-blocks 0..ib, then
                normalize (IEEE reciprocal of the ones-row denominator).
                The diag mask runs here (just before its only consumer) so
                the in-order DVE queue never holds a wait on a future exp."""
                emit_mask(exs[ib], h)
                pv = ps_b.tile([P, D + 1], F32, tag="pv", name=f"pv{h}_{ib}")
                for lb in range(ib + 1):
                    nc.tensor.matmul(
                        pv[:],
                        exs[lb][:, (ib - lb) * P : (ib - lb + 1) * P],
                        v_aug[:, lb, h, :],
                        start=(lb == 0),
                        stop=(lb == ib),
                    )
                nc.vector.reciprocal(rc[:, ib : ib + 1], pv[:, D : D + 1])
                nc.vector.tensor_scalar_mul(
                    asb[:, ib, h % 2, :], pv[:, 0:D], rc[:, ib : ib + 1]
                )
                if h == NH - 1:
                    tp_pending.append(ib)
                elif h % 2 == 1:
                    nc.sync.dma_start_transpose(
                        attnT[:, h // 2, ib * P : (ib + 1) * P], asb[:, ib, :, :]
                    )

            # out-proj is split: MMs in one slot, PSUM->SBUF evac + y DMA a
            # slot later, emitted at the HEAD of the slot so the ready copies
            # never queue behind the burst's exp-gated evac ops on DVE.
            op_pending = {}
            tp_pending = []

            def emit_pe_transposes(asb):
                # head 7's attnT blocks via PE transpose-mode: a PE->DVE chain
                # is shorter than the SP HWDGE hop, keeping the tail tight.
                while tp_pending:
                    ib = tp_pending.pop(0)
                    tp = ps_a.tile([P, P], BF16, tag="fl", name=f"tp{ib}")
                    nc.tensor.transpose(tp[:], asb[:, ib, :, :], ident[:])
                    nc.vector.tensor_copy(
                        attnT[:, JC - 1, ib * P : (ib + 1) * P], tp[:]
                    )

            def emit_outproj_mms(grp, tb, wide=False):
                jcs = (0, 1) if grp == "a" else (2, 3)
                if wide:
                    # tail: the scores rotation is idle by now -- use its
                    # 2-bank tiles so consecutive tail blocks pipeline.
                    po = ps_a.tile([P, 2, 512], F32, tag="sc", name=f"o{grp}_ps{tb}")
                    pos = [po[:, 0, :], po[:, 1, :]]
                else:
                    pos = [
                        ps_a.tile(
                            [P, 512], F32, tag="fl", name=f"o{grp}_ps{tb}_{oc}"
                        )[:]
                        for oc in range(2)
                    ]
                for oc in range(2):
                    for i, jc in enumerate(jcs):
                        nc.tensor.matmul(
                            pos[oc],
                            attnT[:, jc, tb * P : (tb + 1) * P],
                            woutT[:, jc, oc * 512 : (oc + 1) * 512],
                            start=(i == 0),
                            stop=(i == len(jcs) - 1),
                        )
                op_pending[(grp, tb)] = pos

            def emit_outproj_copies(grp, tb):
                pos = op_pending.pop((grp, tb))
                ot = out_pool.tile([P, C], BF16, tag="ot", name=f"ot{grp}{tb}")
                for oc in range(2):
                    nc.vector.tensor_copy(ot[:, oc * 512 : (oc + 1) * 512], pos[oc])
                return ot

            def emit_y_dma(grp, tb, ot):
                # group a runs in heads 5-6 where SP carries the pair-2
                # transposes -- route its y writes through the idle SWDGE.
                eng = nc.gpsimd if grp == "a" else nc.sync
                eng.dma_start((ya_d if grp == "a" else yc_d)[tb], ot[:])

            ot_pending = {}

            def emit_outproj_evac(grp, tb):
                emit_y_dma(grp, tb, emit_outproj_copies(grp, tb))

            # scores run 2 slots ahead of the PV bursts (spilling into the
            # previous head across boundaries); fills slot in between so the
            # in-order PE queue never blocks on DMA.
            pending = {0: {}}
            pending[0][0] = ex00 = open_ex(0, 0)
            emit_scores_chunk(0, 0, ex00, 0)
            emit_qk_super(0, 2, dve=True)
            emit_qk_super(0, 3, dve=True)
            emit_scores_chunk(0, 0, ex00, 1024)
            pending[0][1] = emit_scores_exp(0, 1)
            for h in range(NH):
                if h % 2 == 0:
                    asb = asb_pool.tile(
                        [P, LC, 2, D], BF16, tag="asb", name=f"asb{h // 2}"
                    )
                rc = rc_pool.tile([P, LC], F32, tag="rc", name=f"rc{h}")

                insert_at = {}
                for lb_at, item in inserts[h]:
                    insert_at.setdefault(lb_at, []).append(item)

                exs = pending.pop(h)
                pv_delay = 2 if h == 0 else 1
                for slot in range(LC + pv_delay):
                    # group-a out-proj: heads 5/6 each cover 8 i-blocks.
                    # mms at slots 3..10, PSUM evac + y DMA one slot later.
                    if h in (NH - 3, NH - 2):
                        base = (h - (NH - 3)) * 8
                        if 4 <= slot <= 11:
                            ot_pending["a"] = emit_outproj_copies(
                                "a", base + slot - 4
                            )
                    if h == NH - 1 and 5 <= slot < LC:
                        ot_pending["c"] = emit_outproj_copies("c", slot - 5)
                    for kind, arg in insert_at.get(slot, []):
                        if kind == "v":
                            emit_v_pair(arg)
                        elif kind == "qks":
                            emit_qk_super(*arg)
                        else:
                            emit_qk_pair(*arg)
                    if h == NH - 1:
                        emit_pe_transposes(asb)
                    lb_pv = slot - pv_delay
                    if 0 <= lb_pv:
                        emit_pv_burst(h, lb_pv, exs, rc, asb)
                    if h in (NH - 3, NH - 2):
                        base = (h - (NH - 3)) * 8
                        if 3 <= slot <= 10:
                            emit_outproj_mms("a", base + slot - 3)
                        if 4 <= slot <= 11:
                            emit_y_dma("a", base + slot - 4, ot_pending.pop("a"))
                    if h == NH - 1 and 4 <= slot < LC:
                        emit_outproj_mms("c", slot - 4)
                    if h == NH - 1 and 5 <= slot < LC:
                        emit_y_dma("c", slot - 5, ot_pending.pop("c"))
                    # scores for slot+2 go LAST: their PSUM-slot WAR (waits
                    # exp(slot)) must never block ready work above.
                    nxt = slot + 2
                    if nxt < LC:
                        exs[nxt] = emit_scores_exp(h, nxt)
                    elif nxt < LC + 2 and h + 1 < NH:
                        pending.setdefault(h + 1, {})[nxt - LC] = emit_scores_exp(
                            h + 1, nxt - LC
                        )
                if h == NH - 1:
                    for tb in range(LC - 4, LC):
                        emit_pe_transposes(asb)
                        emit_outproj_mms("c", tb, wide=True)
                        emit_outproj_evac("c", tb - 1)
                    emit_outproj_evac("c", LC - 1)

    nc.compile()
    return nc


_CACHE = {}

# Set by test harnesses to capture a profile; harmless defaults for grading.
TRACE = False
LAST_RESULT = None


def get_program(T=2048):
    if T not in _CACHE:
        _CACHE[T] = build_program(T)
    return _CACHE[T]


def make_in_map(x_b, w_qkv, w_out, hg, T=2048):
    """Host-side shard prep for one core: batch slice x_b [T, C], head group hg."""
    xT = np.ascontiguousarray(x_b.T).astype(NPBF16).reshape(CC, P, T)
    W = np.concatenate(
        [
            w_qkv[hg * J : (hg + 1) * J],
            w_qkv[C + hg * J : C + (hg + 1) * J],
            w_qkv[2 * C + hg * J : 2 * C + (hg + 1) * J],
        ],
        axis=0,
    )  # [3J, C]
    wqkvT = np.ascontiguousarray(W.T).astype(NPBF16).reshape(CC, P, 3 * J)
    Wo = w_out[:, hg * J : (hg + 1) * J]  # [C, J]
    woutT = np.ascontiguousarray(Wo.T).astype(NPBF16).reshape(JC, P, C)
    tri = np.triu(np.ones((P, P), np.float32)).astype(NPBF16)
    ident = np.eye(P, dtype=np.float32).astype(NPBF16)
    return {"xT": xT, "wqkvT": wqkvT, "woutT": woutT, "trimask": tri,
            "ident": ident}


def kernel(x, w_qkv, w_out, b_out):
    x = np.asarray(x, dtype=np.float32)
    w_qkv = np.asarray(w_qkv, dtype=np.float32)
    w_out = np.asarray(w_out, dtype=np.float32)
    b_out = np.asarray(b_out, dtype=np.float32)
    B, T, Cx = x.shape
    assert Cx == C

    nc = get_program(T)
    in_maps = [
        make_in_map(x[core // 2], w_qkv, w_out, core % 2, T) for core in range(8)
    ]
    res = run_bass_kernel_spmd(nc, in_maps, core_ids=list(range(8)), trace=TRACE)
    global LAST_RESULT
    LAST_RESULT = res
    outs = [
        r["ya"].reshape(T, C).astype(np.float32)
        + r["yc"].reshape(T, C).astype(np.float32)
        for r in res.results
    ]
    y = np.stack([outs[2 * b] + outs[2 * b + 1] for b in range(B)])
    return (y + b_out[None, None, :]).astype(np.float32)
